# revision 13
# baseline (speedup 1.0000x reference)
"""Trainium2 Bass kernel for CRF Viterbi decode (nn_CRFLayer).

Problem: inputs [B=512, T=512, K=64] f32 unary potentials, transitions [K, K]
f32.  Output: decode_tags [B, T] int32 (max-plus Viterbi DP + backtrace, with
sequence lengths derived from the nonzero count of the inputs).

Sharding: pure data parallelism over the batch dim: 8 cores x 64 batches.
Each core runs an identical Bass program (SPMD) on its own batch slice.

Per-core layout (partition dim = batch, 64 partitions):
  - x tile [64, T*K]: the input slab; column-block t is overwritten in place
    by alpha_t during the forward pass (pot_t is consumed exactly at step t).
  - forward step t: scores[b,(j,i)] = alpha_{t-1}[b,i] (free-dim broadcast
    over j) + trans_rep[b,(j,i)]; segmented reduce_max over i -> m[b,j];
    alpha_t = m + pot_t (in place).  No freeze masking: the freeze is applied
    post-hoc (alpha_final = alpha_{L-1} via a one-hot time mask; backtrace
    steps with t+1 >= L are identity via a mask in the tag update).
  - backtrace: tags[t] = argmax_i(alpha_t[b,i] + T[i, tags[t+1]]).  The
    T column gather is a PE one-hot matmul; argmax via DVE max/max_index
    (first-occurrence tie semantics, matching jnp.argmax).
"""

import os

import numpy as np

B, T, K = 512, 512, 64
N_CORES = 8
BC = B // N_CORES  # batches per core = 64

_cache = {}


def _build_nc(t_steps: int):
    import concourse.bacc as bacc
    import concourse.mybir as mybir
    from concourse import tile
    from concourse.bass import AP

    fp32 = mybir.dt.float32
    i32 = mybir.dt.int32
    u16 = mybir.dt.uint16
    Alu = mybir.AluOpType
    Axis = mybir.AxisListType

    TK = t_steps * K

    nc = bacc.Bacc("TRN2", target_bir_lowering=False, debug=False,
                   num_devices=N_CORES)

    # ---- DRAM I/O ----
    # x128[p = h*64+b, t*32+c] = inputs[b, t, h*32+c]  (j-half split layout)
    x128_dram = nc.dram_tensor("x128", [2 * BC, t_steps * 32], fp32,
                               kind="ExternalInput")
    # trans128[p, c*64+r] = T[r, (p//64)*32+c]
    trans_rep_dram = nc.dram_tensor("trans_rep", [2 * BC, 32 * K], fp32,
                                    kind="ExternalInput")
    trans_jpart_dram = nc.dram_tensor("trans_jpart", [K, K], fp32,
                                      kind="ExternalInput")
    ident_dram = nc.dram_tensor("ident", [K, K], fp32, kind="ExternalInput")
    # ident128[p, m] = ((p mod 64) == m)
    ident128_dram = nc.dram_tensor("ident128", [2 * BC, K], fp32,
                                   kind="ExternalInput")
    ones_dram = nc.dram_tensor("ones_row", [1, K], fp32, kind="ExternalInput")
    iota_part_dram = nc.dram_tensor("iota_part", [K, K], fp32,
                                    kind="ExternalInput")
    # iota_bt[b, c] = 64*(c+1), c = 0..t_steps (t_steps+1 cols)
    iota_bt_dram = nc.dram_tensor("iota_bt", [BC, t_steps + 1], fp32,
                                  kind="ExternalInput")
    tags_dram = nc.dram_tensor("tags", [BC, t_steps], i32,
                               kind="ExternalOutput")

    TCH = min(32, t_steps)      # time steps per pot chunk
    PCH = TCH * 32              # pot-chunk free elems per partition (1024)
    n_chunks = t_steps // TCH   # 16 at T=512
    assert t_steps % TCH == 0
    CHUNK = TCH * K             # alpha-final chunk elems on x64 (2048)

    with tile.TileContext(nc) as tc:
        with tc.tile_pool(name="sb", bufs=1) as pool, \
             tc.tile_pool(name="ps", bufs=1, space="PSUM") as psum:
            x = pool.tile([BC, TK], fp32)            # alpha archive [b, (t, i)]
            pot_a = pool.tile([2 * BC, PCH], fp32)
            pot_b = pool.tile([2 * BC, PCH], fp32)
            trans_rep = pool.tile([2 * BC, 32 * K], fp32)
            trans_jpart = pool.tile([K, K], fp32)
            ident = pool.tile([K, K], fp32)
            ident128 = pool.tile([2 * BC, K], fp32)
            ones_row = pool.tile([1, K], fp32)
            iota_part = pool.tile([K, K], fp32)
            iota_bt = pool.tile([BC, t_steps + 1], fp32)

            scores = pool.tile([2 * BC, 32 * K], fp32)
            m128 = pool.tile([2 * BC, 32], fp32)
            ah0 = pool.tile([2 * BC, 32], fp32)
            ah1 = pool.tile([2 * BC, 32], fp32)
            alpha_halves = [ah0, ah1]
            cnt_tmp = pool.tile([2 * BC, PCH], fp32)
            counts = pool.tile([2 * BC, n_chunks], fp32)
            cnt_col = pool.tile([2 * BC, 1], fp32)
            c_total = pool.tile([BC, 1], fp32)
            selA = pool.tile([BC, t_steps + 1], fp32)
            lsel = pool.tile([BC, t_steps], fp32)
            btmask2 = pool.tile([BC, t_steps], fp32)
            partials = pool.tile([BC, (TK // CHUNK) * K], fp32)
            alpha_fin = pool.tile([BC, K], fp32)
            mx8 = pool.tile([BC, 8], fp32)
            idx8 = pool.tile([BC, 8], u16)
            dcol = pool.tile([BC, 1], fp32)
            tagsf = pool.tile([BC, t_steps], fp32)
            tags_i = pool.tile([BC, t_steps], i32)
            onehot_T = pool.tile([K, K], fp32)
            tag_row = pool.tile([1, K], fp32)
            score_bt = pool.tile([BC, K], fp32)
            l0fix = pool.tile([BC, 1], fp32)

            arp0 = psum.tile([2 * BC, K], fp32)
            arp1 = psum.tile([2 * BC, K], fp32)
            alpha_rep_ps = [arp0, arp1]
            cnt_hi_ps = psum.tile([BC, 1], fp32)
            tag_rowT_ps = psum.tile([1, K], fp32)
            tag_bcast_ps = psum.tile([K, K], fp32)
            t_row_ps = psum.tile([BC, K], fp32)

            # ---- load constants ----
            nc.sync.dma_start(out=trans_rep[:], in_=trans_rep_dram[:])
            nc.sync.dma_start(out=trans_jpart[:], in_=trans_jpart_dram[:])
            nc.sync.dma_start(out=ident[:], in_=ident_dram[:])
            nc.sync.dma_start(out=ident128[:], in_=ident128_dram[:])
            nc.sync.dma_start(out=ones_row[:], in_=ones_dram[:])
            nc.sync.dma_start(out=iota_part[:], in_=iota_part_dram[:])
            nc.sync.dma_start(out=iota_bt[:], in_=iota_bt_dram[:])

            pots = [pot_a, pot_b]

            def pot_slice(t):
                return pots[(t // TCH) % 2][:, (t % TCH) * 32:
                                            (t % TCH) * 32 + 32]

            def load_chunk(c):
                nc.sync.dma_start(out=pots[c % 2][:],
                                  in_=x128_dram[:, c * PCH:(c + 1) * PCH])

            def count_chunk(c):
                nc.vector.tensor_scalar(cnt_tmp[:], pots[c % 2][:], 0.0, None,
                                        op0=Alu.not_equal)
                nc.vector.tensor_reduce(
                    counts[:, c:c + 1],
                    cnt_tmp[:].rearrange("p (a b) -> p a b", b=PCH),
                    axis=Axis.XY, op=Alu.add)

            def exchange(t, src):
                """alpha_rep_ps[t%2] <- replicate src [128, 32] across halves;
                side-DMA full alpha row block into the x archive (b-layout)."""
                par = t % 2
                dst = alpha_rep_ps[par]
                nc.vector.tensor_copy(dst[0:BC, 0:32], src[0:BC, :])
                nc.vector.tensor_copy(dst[BC:2 * BC, 32:64], src[BC:2 * BC, :])
                nc.tensor.matmul(dst[0:BC, 32:64], ident128[BC:2 * BC, :],
                                 src[BC:2 * BC, :], start=True, stop=True)
                nc.tensor.matmul(dst[BC:2 * BC, 0:32], ident128[0:BC, :],
                                 src[0:BC, :], start=True, stop=True)
                nc.sync.dma_start(out=x[:, t * K:t * K + 32],
                                  in_=src[0:BC, :])
                nc.sync.dma_start(out=x[:, t * K + 32:(t + 1) * K],
                                  in_=src[BC:2 * BC, :])

            # ---- chunk 0 + init ----
            load_chunk(0)
            if n_chunks > 1:
                load_chunk(1)
            count_chunk(0)
            exchange(0, pot_slice(0))

            # ---- forward DP (unmasked) ----
            scores3 = scores[:].rearrange("p (j i) -> p j i", i=K)
            trans3 = trans_rep[:].rearrange("p (j i) -> p j i", i=K)
            for t in range(1, t_steps):
                ch = t // TCH
                if t % TCH == 0:
                    if ch + 1 < n_chunks:
                        load_chunk(ch + 1)
                    count_chunk(ch)
                a_prev = alpha_rep_ps[(t - 1) % 2]
                a_bc = a_prev[:].unsqueeze(1).to_broadcast([2 * BC, 32, K])
                nc.vector.tensor_add(scores3, a_bc, trans3)
                nc.vector.tensor_reduce(m128[:], scores3, axis=Axis.X,
                                        op=Alu.max)
                ah = alpha_halves[t % 2]
                nc.vector.tensor_add(ah[:], m128[:], pot_slice(t))
                exchange(t, ah)

            # ---- seq lens: c_total[b] = sum of both partition halves ----
            nc.vector.tensor_reduce(cnt_col[:],
                                    counts[:].rearrange("p (a b) -> p a b",
                                                        b=n_chunks),
                                    axis=Axis.XY, op=Alu.add)
            nc.tensor.matmul(cnt_hi_ps[:], ident128[BC:2 * BC, :],
                             cnt_col[BC:2 * BC, :], start=True, stop=True)
            nc.vector.tensor_add(c_total[:], cnt_col[0:BC, :], cnt_hi_ps[:])

            # selA[b, c] = (64*(c+1) <= c_total)   [c = 0..t_steps]
            nc.vector.tensor_scalar(selA[:], iota_bt[:], c_total[:, 0:1], None,
                                    op0=Alu.is_le)
            # lsel[b, t] = (t == L-1):  selA[t] - selA[t+1]
            nc.vector.tensor_sub(lsel[:], selA[:, 0:t_steps],
                                 selA[:, 1:t_steps + 1])
            # clamp L >= 1: if c_total < 64 (L == 0), select t = 0
            nc.vector.tensor_scalar(l0fix[:], c_total[:], 64.0, None,
                                    op0=Alu.is_lt)
            nc.vector.tensor_add(lsel[:, 0:1], lsel[:, 0:1], l0fix[:])
            # btmask2[b, t] = (64*(t+1) <= c_total)  (= t+1 < L)
            nc.vector.tensor_copy(btmask2[:], selA[:, 0:t_steps])

            # ---- alpha_final = sum_t alpha_t * lsel_t  (exact one-hot sum) ----
            n_fchunks = TK // CHUNK
            for c in range(n_fchunks):
                # x view [b, (i outer, t inner strided)]
                xv = AP(x.tensor, c * CHUNK,
                        [[x.tensor.shape[1], BC], [1, K], [K, TCH]])
                lv = AP(lsel.tensor, c * TCH,
                        [[lsel.tensor.shape[1], BC], [0, K], [1, TCH]])
                sv = scores[0:BC, :CHUNK].rearrange("p (i t) -> p i t", t=TCH)
                nc.vector.tensor_mul(sv, xv, lv)
                nc.vector.tensor_reduce(
                    partials[:, c * K:(c + 1) * K],
                    sv, axis=Axis.X, op=Alu.add)
            pv = AP(partials.tensor, 0,
                    [[partials.tensor.shape[1], BC], [1, K], [K, n_fchunks]])
            nc.vector.tensor_reduce(alpha_fin[:], pv, axis=Axis.X, op=Alu.add)

            # ---- last tag ----
            nc.vector.max(out=mx8[:], in_=alpha_fin[:])
            nc.vector.max_index(out=idx8[:], in_max=mx8[:],
                                in_values=alpha_fin[:])
            nc.vector.tensor_copy(tagsf[:, t_steps - 1:t_steps], idx8[:, 0:1])

            # ---- backtrace ----
            for t in range(t_steps - 2, -1, -1):
                tag_col = tagsf[:, t + 1:t + 2]
                # tag_row[0, b] = tag[b]
                nc.tensor.transpose(tag_rowT_ps[:], tag_col, ident[:])
                nc.scalar.copy(tag_row[:], tag_rowT_ps[:])
                # tag_bcast[j, b] = tag[b]
                nc.tensor.matmul(tag_bcast_ps[:], ones_row[:], tag_row[:],
                                 start=True, stop=True)
                # onehot_T[j, b] = (j == tag[b])
                nc.vector.tensor_tensor(onehot_T[:], iota_part[:],
                                        tag_bcast_ps[:], op=Alu.is_equal)
                # t_row[b, i] = T[i, tag[b]]
                nc.tensor.matmul(t_row_ps[:], onehot_T[:], trans_jpart[:],
                                 start=True, stop=True)
                # score_bt[b, i] = alpha_t[b, i] + T[i, tag[b]]
                nc.vector.tensor_add(score_bt[:], x[:, t * K:(t + 1) * K],
                                     t_row_ps[:])
                nc.vector.max(out=mx8[:], in_=score_bt[:])
                nc.vector.max_index(out=idx8[:], in_max=mx8[:],
                                    in_values=score_bt[:])
                # tags[t] = tag + mask*(idx - tag)
                nc.vector.tensor_sub(dcol[:], idx8[:, 0:1], tag_col)
                nc.vector.scalar_tensor_tensor(
                    out=tagsf[:, t:t + 1], in0=dcol[:],
                    scalar=btmask2[:, t + 1:t + 2], in1=tag_col,
                    op0=Alu.mult, op1=Alu.add)

            # ---- emit ----
            nc.vector.tensor_copy(tags_i[:], tagsf[:])
            nc.sync.dma_start(out=tags_dram[:], in_=tags_i[:])

    nc.finalize()
    return nc


def _host_tables(transitions: np.ndarray, t_steps: int):
    tt = np.ascontiguousarray(transitions.T.astype(np.float32))  # [j, i]
    # trans_rep[p, c*64+r] = T[r, (p//64)*32 + c]; same for all b within a half
    half0 = tt[0:32, :].reshape(1, 32 * K)
    half1 = tt[32:64, :].reshape(1, 32 * K)
    trans_rep = np.concatenate([
        np.broadcast_to(half0, (BC, 32 * K)),
        np.broadcast_to(half1, (BC, 32 * K))], axis=0).copy()
    trans_jpart = tt.copy()  # [j, i] = T[i, j]
    ident = np.eye(K, dtype=np.float32)
    ident128 = np.concatenate([np.eye(K), np.eye(K)],
                              axis=0).astype(np.float32)
    ones_row = np.ones((1, K), dtype=np.float32)
    iota_part = np.broadcast_to(
        np.arange(K, dtype=np.float32)[:, None], (K, K)).copy()
    iota_bt = np.broadcast_to(
        (64.0 * np.arange(1, t_steps + 2, dtype=np.float32))[None, :],
        (BC, t_steps + 1)).copy()
    return {
        "trans_rep": trans_rep,
        "trans_jpart": trans_jpart,
        "ident": ident,
        "ident128": ident128,
        "ones_row": ones_row,
        "iota_part": iota_part,
        "iota_bt": iota_bt,
    }


def _x128_of(x_core: np.ndarray, t_steps: int) -> np.ndarray:
    """[BC, T, K] -> [128, T*32] with p = h*64+b, cols (t, c), j = h*32+c."""
    return np.ascontiguousarray(
        x_core.reshape(BC, t_steps, 2, 32).transpose(2, 0, 1, 3)
        .reshape(2 * BC, t_steps * 32))


class _Runner:
    """Caches the jitted 8-core sharded executable for a built nc.

    Mirrors concourse.bass2jax.run_bass_via_pjrt, but keeps a stable jitted
    callable so repeat calls skip retracing/recompiling.
    """

    def __init__(self, nc):
        import jax
        import concourse.mybir as mybir
        from concourse import bass2jax
        from jax.sharding import Mesh, PartitionSpec
        from jax.experimental.shard_map import shard_map

        bass2jax.install_neuronx_cc_hook()
        assert nc.dbg_addr is None
        partition_name = (nc.partition_id_tensor.name
                          if nc.partition_id_tensor else None)

        in_names, out_names, out_avals = [], [], []
        for alloc in nc.m.functions[0].allocations:
            if not isinstance(alloc, mybir.MemoryLocationSet):
                continue
            name = alloc.memorylocations[0].name
            if alloc.kind == "ExternalInput":
                if name != partition_name:
                    in_names.append(name)
            elif alloc.kind == "ExternalOutput":
                out_names.append(name)
                out_avals.append(jax.core.ShapedArray(
                    tuple(alloc.tensor_shape), mybir.dt.np(alloc.dtype)))
        self.in_names = list(in_names)
        self.out_names = out_names
        self.out_avals = out_avals
        n_params = len(in_names)
        n_outs = len(out_avals)
        all_in_names = in_names + out_names
        if partition_name is not None:
            all_in_names = all_in_names + [partition_name]

        def _body(*args):
            operands = list(args)
            if partition_name is not None:
                operands.append(bass2jax.partition_id_tensor())
            outs = bass2jax._bass_exec_p.bind(
                *operands,
                out_avals=tuple(out_avals),
                in_names=tuple(all_in_names),
                out_names=tuple(out_names),
                lowering_input_output_aliases=(),
                sim_require_finite=True,
                sim_require_nnan=True,
                nc=nc,
            )
            return tuple(outs)

        devices = jax.devices()[:N_CORES]
        mesh = Mesh(np.asarray(devices), ("core",))
        in_specs = (PartitionSpec("core"),) * (n_params + n_outs)
        out_specs = (PartitionSpec("core"),) * n_outs
        self._fn = jax.jit(
            shard_map(_body, mesh=mesh, in_specs=in_specs,
                      out_specs=out_specs, check_rep=False),
            donate_argnums=tuple(range(n_params, n_params + n_outs)),
            keep_unused=True,
        )

    def __call__(self, concat_in):
        zeros = [np.zeros((N_CORES * a.shape[0], *a.shape[1:]), a.dtype)
                 for a in self.out_avals]
        out = self._fn(*concat_in, *zeros)
        return {name: np.asarray(out[i]) for i, name in
                enumerate(self.out_names)}


def _get_runner(t_steps: int) -> "_Runner":
    key = t_steps
    if key not in _cache:
        _cache[key] = _Runner(_build_nc(t_steps))
    return _cache[key]


def _concat_inputs(runner, x_full, tables):
    t_steps = x_full.shape[1] // K
    per_core = []
    for c in range(N_CORES):
        xc = x_full[c * BC:(c + 1) * BC].reshape(BC, t_steps, K)
        m = {"x128": _x128_of(xc, t_steps)}
        m.update(tables)
        per_core.append(m)
    return [np.concatenate([per_core[c][n] for c in range(N_CORES)], axis=0)
            for n in runner.in_names]


def _run_spmd_fallback(t_steps, x_full, tables):
    """Non-PJRT path (native NRT): run via bass_utils.run_bass_kernel_spmd."""
    from concourse.bass_utils import run_bass_kernel_spmd
    key = ("nc", t_steps)
    if key not in _cache:
        _cache[key] = _build_nc(t_steps)
    nc = _cache[key]
    in_maps = []
    for c in range(N_CORES):
        xc = x_full[c * BC:(c + 1) * BC].reshape(BC, t_steps, K)
        m = {"x128": _x128_of(xc, t_steps)}
        m.update(tables)
        in_maps.append(m)
    res = run_bass_kernel_spmd(nc, in_maps, core_ids=list(range(N_CORES)))
    return np.concatenate([r["tags"] for r in res.results], axis=0)


def kernel(inputs: np.ndarray, transitions: np.ndarray) -> np.ndarray:
    t_steps = inputs.shape[1]
    tables = _host_tables(transitions, t_steps)
    x_full = np.ascontiguousarray(
        inputs.reshape(B, t_steps * K).astype(np.float32))
    try:
        runner = _get_runner(t_steps)
        concat_in = _concat_inputs(runner, x_full, tables)
        res = runner(concat_in)
        out = res["tags"].reshape(B, t_steps)
    except Exception:
        out = _run_spmd_fallback(t_steps, x_full, tables)
    return out.astype(np.int32)


kernel.last_exec_time_ns = None


# revision 14
# speedup vs baseline: 1.5572x; 1.5572x over previous
"""Trainium2 Bass kernel for CRF Viterbi decode (nn_CRFLayer).

Problem: inputs [B=512, T=512, K=64] f32 unary potentials, transitions [K, K]
f32.  Output: decode_tags [B, T] int32 (max-plus Viterbi DP + backtrace, with
sequence lengths derived from the nonzero count of the inputs).

Sharding: pure data parallelism over the batch dim: 8 cores x 64 batches.
Each core runs an identical Bass program (SPMD) on its own batch slice.

Per-core layout (partition dim = batch, 64 partitions):
  - x tile [64, T*K]: the input slab; column-block t is overwritten in place
    by alpha_t during the forward pass (pot_t is consumed exactly at step t).
  - forward step t: scores[b,(j,i)] = alpha_{t-1}[b,i] (free-dim broadcast
    over j) + trans_rep[b,(j,i)]; segmented reduce_max over i -> m[b,j];
    alpha_t = m + pot_t (in place).  No freeze masking: the freeze is applied
    post-hoc (alpha_final = alpha_{L-1} via a one-hot time mask; backtrace
    steps with t+1 >= L are identity via a mask in the tag update).
  - backtrace: tags[t] = argmax_i(alpha_t[b,i] + T[i, tags[t+1]]).  The
    T column gather is a PE one-hot matmul; argmax via DVE max/max_index
    (first-occurrence tie semantics, matching jnp.argmax).
"""

import os

import numpy as np

B, T, K = 512, 512, 64
N_CORES = 8
BC = B // N_CORES  # batches per core = 64

_cache = {}


def _build_nc(t_steps: int):
    import concourse.bacc as bacc
    import concourse.mybir as mybir
    from concourse import tile
    from concourse.bass import AP

    fp32 = mybir.dt.float32
    i32 = mybir.dt.int32
    u16 = mybir.dt.uint16
    Alu = mybir.AluOpType
    Axis = mybir.AxisListType

    TK = t_steps * K

    nc = bacc.Bacc("TRN2", target_bir_lowering=False, debug=False,
                   num_devices=N_CORES)

    # ---- DRAM I/O ----
    # x128[p = h*64+b, t*32+c] = inputs[b, t, h*32+c]  (j-half split layout)
    x128_dram = nc.dram_tensor("x128", [2 * BC, t_steps * 32], fp32,
                               kind="ExternalInput")
    # trans128[p, c*64+r] = T[r, (p//64)*32+c]
    trans_rep_dram = nc.dram_tensor("trans_rep", [2 * BC, 32 * K], fp32,
                                    kind="ExternalInput")
    trans_jpart_dram = nc.dram_tensor("trans_jpart", [K, K], fp32,
                                      kind="ExternalInput")
    ident_dram = nc.dram_tensor("ident", [K, K], fp32, kind="ExternalInput")
    # ident128[p, m] = ((p mod 64) == m)
    ident128_dram = nc.dram_tensor("ident128", [2 * BC, K], fp32,
                                   kind="ExternalInput")
    ones_dram = nc.dram_tensor("ones_row", [1, K], fp32, kind="ExternalInput")
    iota_part_dram = nc.dram_tensor("iota_part", [K, K], fp32,
                                    kind="ExternalInput")
    # iota_bt[b, c] = 64*(c+1), c = 0..t_steps (t_steps+1 cols)
    iota_bt_dram = nc.dram_tensor("iota_bt", [BC, t_steps + 1], fp32,
                                  kind="ExternalInput")
    tags_dram = nc.dram_tensor("tags", [BC, t_steps], i32,
                               kind="ExternalOutput")

    TCH = min(32, t_steps)      # time steps per pot chunk
    PCH = TCH * 32              # pot-chunk free elems per partition (1024)
    n_chunks = t_steps // TCH   # 16 at T=512
    assert t_steps % TCH == 0
    CHUNK = TCH * K             # alpha-final chunk elems on x64 (2048)

    with tile.TileContext(nc) as tc:
        with tc.tile_pool(name="sb", bufs=1) as pool, \
             tc.tile_pool(name="ps", bufs=1, space="PSUM") as psum:
            x = pool.tile([BC, TK], fp32)            # alpha archive [b, (t, i)]
            pot_a = pool.tile([2 * BC, PCH], fp32)
            pot_b = pool.tile([2 * BC, PCH], fp32)
            trans_rep = pool.tile([2 * BC, 32 * K], fp32)
            trans_jpart = pool.tile([K, K], fp32)
            ident = pool.tile([K, K], fp32)
            ident128 = pool.tile([2 * BC, K], fp32)
            ones_row = pool.tile([1, K], fp32)
            iota_part = pool.tile([K, K], fp32)
            iota_bt = pool.tile([BC, t_steps + 1], fp32)

            scores = pool.tile([2 * BC, 32 * K], fp32)
            m128 = pool.tile([2 * BC, 32], fp32)
            ah0 = pool.tile([2 * BC, 32], fp32)
            ah1 = pool.tile([2 * BC, 32], fp32)
            alpha_halves = [ah0, ah1]
            cnt_tmp = pool.tile([2 * BC, PCH], fp32)
            counts = pool.tile([2 * BC, n_chunks], fp32)
            cnt_col = pool.tile([2 * BC, 1], fp32)
            c_total = pool.tile([BC, 1], fp32)
            selA = pool.tile([BC, t_steps + 1], fp32)
            lsel = pool.tile([BC, t_steps], fp32)
            btmask2 = pool.tile([BC, t_steps], fp32)
            partials = pool.tile([BC, (TK // CHUNK) * K], fp32)
            alpha_fin = pool.tile([BC, K], fp32)
            mx8 = pool.tile([BC, 8], fp32)
            idx8 = pool.tile([BC, 8], u16)
            dcol = pool.tile([BC, 1], fp32)
            tagsf = pool.tile([BC, t_steps], fp32)
            tags_i = pool.tile([BC, t_steps], i32)
            onehot_T = pool.tile([K, K], fp32)
            tag_row = pool.tile([1, K], fp32)
            score_bt = pool.tile([BC, K], fp32)
            l0fix = pool.tile([BC, 1], fp32)

            arp0 = psum.tile([2 * BC, K], fp32)
            arp1 = psum.tile([2 * BC, K], fp32)
            alpha_rep_ps = [arp0, arp1]
            cnt_hi_ps = psum.tile([BC, 1], fp32)
            tag_rowT_ps = psum.tile([1, K], fp32)
            tag_bcast_ps = psum.tile([K, K], fp32)
            t_row_ps = psum.tile([BC, K], fp32)

            # ---- load constants ----
            nc.sync.dma_start(out=trans_rep[:], in_=trans_rep_dram[:])
            nc.sync.dma_start(out=trans_jpart[:], in_=trans_jpart_dram[:])
            nc.sync.dma_start(out=ident[:], in_=ident_dram[:])
            nc.sync.dma_start(out=ident128[:], in_=ident128_dram[:])
            nc.sync.dma_start(out=ones_row[:], in_=ones_dram[:])
            nc.sync.dma_start(out=iota_part[:], in_=iota_part_dram[:])
            nc.sync.dma_start(out=iota_bt[:], in_=iota_bt_dram[:])

            pots = [pot_a, pot_b]

            def pot_slice(t):
                return pots[(t // TCH) % 2][:, (t % TCH) * 32:
                                            (t % TCH) * 32 + 32]

            def load_chunk(c):
                nc.sync.dma_start(out=pots[c % 2][:],
                                  in_=x128_dram[:, c * PCH:(c + 1) * PCH])

            def count_chunk(c):
                nc.vector.tensor_scalar(cnt_tmp[:], pots[c % 2][:], 0.0, None,
                                        op0=Alu.not_equal)
                nc.vector.tensor_reduce(
                    counts[:, c:c + 1],
                    cnt_tmp[:].rearrange("p (a b) -> p a b", b=PCH),
                    axis=Axis.XY, op=Alu.add)

            def exchange(t, src):
                """alpha_rep_ps[t%2] <- replicate src [128, 32] across halves;
                side-DMA full alpha row block into the x archive (b-layout)."""
                par = t % 2
                dst = alpha_rep_ps[par]
                nc.vector.tensor_copy(dst[0:BC, 0:32], src[0:BC, :])
                nc.vector.tensor_copy(dst[BC:2 * BC, 32:64], src[BC:2 * BC, :])
                nc.tensor.matmul(dst[0:BC, 32:64], ident128[BC:2 * BC, :],
                                 src[BC:2 * BC, :], start=True, stop=True)
                nc.tensor.matmul(dst[BC:2 * BC, 0:32], ident128[0:BC, :],
                                 src[0:BC, :], start=True, stop=True)
                nc.sync.dma_start(out=x[:, t * K:t * K + 32],
                                  in_=src[0:BC, :])
                nc.sync.dma_start(out=x[:, t * K + 32:(t + 1) * K],
                                  in_=src[BC:2 * BC, :])

            # ---- chunk 0 + init ----
            load_chunk(0)
            if n_chunks > 1:
                load_chunk(1)
            count_chunk(0)
            exchange(0, pot_slice(0))

            # ---- forward DP (unmasked) ----
            scores3 = scores[:].rearrange("p (j i) -> p j i", i=K)
            trans3 = trans_rep[:].rearrange("p (j i) -> p j i", i=K)
            for t in range(1, t_steps):
                ch = t // TCH
                if t % TCH == 0:
                    if ch + 1 < n_chunks:
                        load_chunk(ch + 1)
                    count_chunk(ch)
                a_prev = alpha_rep_ps[(t - 1) % 2]
                a_bc = a_prev[:].unsqueeze(1).to_broadcast([2 * BC, 32, K])
                nc.vector.tensor_add(scores3, a_bc, trans3)
                nc.vector.tensor_reduce(m128[:], scores3, axis=Axis.X,
                                        op=Alu.max)
                ah = alpha_halves[t % 2]
                nc.vector.tensor_add(ah[:], m128[:], pot_slice(t))
                exchange(t, ah)

            # ---- seq lens: c_total[b] = sum of both partition halves ----
            nc.vector.tensor_reduce(cnt_col[:],
                                    counts[:].rearrange("p (a b) -> p a b",
                                                        b=n_chunks),
                                    axis=Axis.XY, op=Alu.add)
            nc.tensor.matmul(cnt_hi_ps[:], ident128[BC:2 * BC, :],
                             cnt_col[BC:2 * BC, :], start=True, stop=True)
            nc.vector.tensor_add(c_total[:], cnt_col[0:BC, :], cnt_hi_ps[:])

            # selA[b, c] = (64*(c+1) <= c_total)   [c = 0..t_steps]
            nc.vector.tensor_scalar(selA[:], iota_bt[:], c_total[:, 0:1], None,
                                    op0=Alu.is_le)
            # lsel[b, t] = (t == L-1):  selA[t] - selA[t+1]
            nc.vector.tensor_sub(lsel[:], selA[:, 0:t_steps],
                                 selA[:, 1:t_steps + 1])
            # clamp L >= 1: if c_total < 64 (L == 0), select t = 0
            nc.vector.tensor_scalar(l0fix[:], c_total[:], 64.0, None,
                                    op0=Alu.is_lt)
            nc.vector.tensor_add(lsel[:, 0:1], lsel[:, 0:1], l0fix[:])
            # btmask2[b, t] = (64*(t+1) <= c_total)  (= t+1 < L)
            nc.vector.tensor_copy(btmask2[:], selA[:, 0:t_steps])

            # ---- alpha_final = sum_t alpha_t * lsel_t  (exact one-hot sum) ----
            n_fchunks = TK // CHUNK
            for c in range(n_fchunks):
                # x view [b, (i outer, t inner strided)]
                xv = AP(x.tensor, c * CHUNK,
                        [[x.tensor.shape[1], BC], [1, K], [K, TCH]])
                lv = AP(lsel.tensor, c * TCH,
                        [[lsel.tensor.shape[1], BC], [0, K], [1, TCH]])
                sv = scores[0:BC, :CHUNK].rearrange("p (i t) -> p i t", t=TCH)
                nc.vector.tensor_mul(sv, xv, lv)
                nc.vector.tensor_reduce(
                    partials[:, c * K:(c + 1) * K],
                    sv, axis=Axis.X, op=Alu.add)
            pv = AP(partials.tensor, 0,
                    [[partials.tensor.shape[1], BC], [1, K], [K, n_fchunks]])
            nc.vector.tensor_reduce(alpha_fin[:], pv, axis=Axis.X, op=Alu.add)

            # ---- last tag ----
            nc.vector.max(out=mx8[:], in_=alpha_fin[:])
            nc.vector.max_index(out=idx8[:], in_max=mx8[:],
                                in_values=alpha_fin[:])
            nc.vector.tensor_copy(tagsf[:, t_steps - 1:t_steps], idx8[:, 0:1])

            # ---- backtrace ----
            for t in range(t_steps - 2, -1, -1):
                tag_col = tagsf[:, t + 1:t + 2]
                # tag_row[0, b] = tag[b]
                nc.tensor.transpose(tag_rowT_ps[:], tag_col, ident[:])
                nc.vector.tensor_copy(tag_row[:], tag_rowT_ps[:])
                # tag_bcast[j, b] = tag[b]
                nc.tensor.matmul(tag_bcast_ps[:], ones_row[:], tag_row[:],
                                 start=True, stop=True)
                # onehot_T[j, b] = (j == tag[b])
                nc.vector.tensor_tensor(onehot_T[:], iota_part[:],
                                        tag_bcast_ps[:], op=Alu.is_equal)
                # t_row[b, i] = T[i, tag[b]]
                nc.tensor.matmul(t_row_ps[:], onehot_T[:], trans_jpart[:],
                                 start=True, stop=True)
                # score_bt[b, i] = alpha_t[b, i] + T[i, tag[b]]
                nc.vector.tensor_add(score_bt[:], x[:, t * K:(t + 1) * K],
                                     t_row_ps[:])
                nc.vector.max(out=mx8[:], in_=score_bt[:])
                nc.vector.max_index(out=idx8[:], in_max=mx8[:],
                                    in_values=score_bt[:])
                # tags[t] = tag + mask*(idx - tag)
                nc.vector.tensor_sub(dcol[:], idx8[:, 0:1], tag_col)
                nc.vector.scalar_tensor_tensor(
                    out=tagsf[:, t:t + 1], in0=dcol[:],
                    scalar=btmask2[:, t + 1:t + 2], in1=tag_col,
                    op0=Alu.mult, op1=Alu.add)

            # ---- emit ----
            nc.vector.tensor_copy(tags_i[:], tagsf[:])
            nc.sync.dma_start(out=tags_dram[:], in_=tags_i[:])

    nc.finalize()
    return nc


def _host_tables(transitions: np.ndarray, t_steps: int):
    tt = np.ascontiguousarray(transitions.T.astype(np.float32))  # [j, i]
    # trans_rep[p, c*64+r] = T[r, (p//64)*32 + c]; same for all b within a half
    half0 = tt[0:32, :].reshape(1, 32 * K)
    half1 = tt[32:64, :].reshape(1, 32 * K)
    trans_rep = np.concatenate([
        np.broadcast_to(half0, (BC, 32 * K)),
        np.broadcast_to(half1, (BC, 32 * K))], axis=0).copy()
    trans_jpart = tt.copy()  # [j, i] = T[i, j]
    ident = np.eye(K, dtype=np.float32)
    ident128 = np.concatenate([np.eye(K), np.eye(K)],
                              axis=0).astype(np.float32)
    ones_row = np.ones((1, K), dtype=np.float32)
    iota_part = np.broadcast_to(
        np.arange(K, dtype=np.float32)[:, None], (K, K)).copy()
    iota_bt = np.broadcast_to(
        (64.0 * np.arange(1, t_steps + 2, dtype=np.float32))[None, :],
        (BC, t_steps + 1)).copy()
    return {
        "trans_rep": trans_rep,
        "trans_jpart": trans_jpart,
        "ident": ident,
        "ident128": ident128,
        "ones_row": ones_row,
        "iota_part": iota_part,
        "iota_bt": iota_bt,
    }


def _x128_of(x_core: np.ndarray, t_steps: int) -> np.ndarray:
    """[BC, T, K] -> [128, T*32] with p = h*64+b, cols (t, c), j = h*32+c."""
    return np.ascontiguousarray(
        x_core.reshape(BC, t_steps, 2, 32).transpose(2, 0, 1, 3)
        .reshape(2 * BC, t_steps * 32))


class _Runner:
    """Caches the jitted 8-core sharded executable for a built nc.

    Mirrors concourse.bass2jax.run_bass_via_pjrt, but keeps a stable jitted
    callable so repeat calls skip retracing/recompiling.
    """

    def __init__(self, nc):
        import jax
        import concourse.mybir as mybir
        from concourse import bass2jax
        from jax.sharding import Mesh, PartitionSpec
        from jax.experimental.shard_map import shard_map

        bass2jax.install_neuronx_cc_hook()
        assert nc.dbg_addr is None
        partition_name = (nc.partition_id_tensor.name
                          if nc.partition_id_tensor else None)

        in_names, out_names, out_avals = [], [], []
        for alloc in nc.m.functions[0].allocations:
            if not isinstance(alloc, mybir.MemoryLocationSet):
                continue
            name = alloc.memorylocations[0].name
            if alloc.kind == "ExternalInput":
                if name != partition_name:
                    in_names.append(name)
            elif alloc.kind == "ExternalOutput":
                out_names.append(name)
                out_avals.append(jax.core.ShapedArray(
                    tuple(alloc.tensor_shape), mybir.dt.np(alloc.dtype)))
        self.in_names = list(in_names)
        self.out_names = out_names
        self.out_avals = out_avals
        n_params = len(in_names)
        n_outs = len(out_avals)
        all_in_names = in_names + out_names
        if partition_name is not None:
            all_in_names = all_in_names + [partition_name]

        def _body(*args):
            operands = list(args)
            if partition_name is not None:
                operands.append(bass2jax.partition_id_tensor())
            outs = bass2jax._bass_exec_p.bind(
                *operands,
                out_avals=tuple(out_avals),
                in_names=tuple(all_in_names),
                out_names=tuple(out_names),
                lowering_input_output_aliases=(),
                sim_require_finite=True,
                sim_require_nnan=True,
                nc=nc,
            )
            return tuple(outs)

        devices = jax.devices()[:N_CORES]
        mesh = Mesh(np.asarray(devices), ("core",))
        in_specs = (PartitionSpec("core"),) * (n_params + n_outs)
        out_specs = (PartitionSpec("core"),) * n_outs
        self._fn = jax.jit(
            shard_map(_body, mesh=mesh, in_specs=in_specs,
                      out_specs=out_specs, check_rep=False),
            donate_argnums=tuple(range(n_params, n_params + n_outs)),
            keep_unused=True,
        )

    def __call__(self, concat_in):
        zeros = [np.zeros((N_CORES * a.shape[0], *a.shape[1:]), a.dtype)
                 for a in self.out_avals]
        out = self._fn(*concat_in, *zeros)
        return {name: np.asarray(out[i]) for i, name in
                enumerate(self.out_names)}


def _get_runner(t_steps: int) -> "_Runner":
    key = t_steps
    if key not in _cache:
        _cache[key] = _Runner(_build_nc(t_steps))
    return _cache[key]


def _concat_inputs(runner, x_full, tables):
    t_steps = x_full.shape[1] // K
    per_core = []
    for c in range(N_CORES):
        xc = x_full[c * BC:(c + 1) * BC].reshape(BC, t_steps, K)
        m = {"x128": _x128_of(xc, t_steps)}
        m.update(tables)
        per_core.append(m)
    return [np.concatenate([per_core[c][n] for c in range(N_CORES)], axis=0)
            for n in runner.in_names]


def _run_spmd_fallback(t_steps, x_full, tables):
    """Non-PJRT path (native NRT): run via bass_utils.run_bass_kernel_spmd."""
    from concourse.bass_utils import run_bass_kernel_spmd
    key = ("nc", t_steps)
    if key not in _cache:
        _cache[key] = _build_nc(t_steps)
    nc = _cache[key]
    in_maps = []
    for c in range(N_CORES):
        xc = x_full[c * BC:(c + 1) * BC].reshape(BC, t_steps, K)
        m = {"x128": _x128_of(xc, t_steps)}
        m.update(tables)
        in_maps.append(m)
    res = run_bass_kernel_spmd(nc, in_maps, core_ids=list(range(N_CORES)))
    return np.concatenate([r["tags"] for r in res.results], axis=0)


def kernel(inputs: np.ndarray, transitions: np.ndarray) -> np.ndarray:
    t_steps = inputs.shape[1]
    tables = _host_tables(transitions, t_steps)
    x_full = np.ascontiguousarray(
        inputs.reshape(B, t_steps * K).astype(np.float32))
    try:
        runner = _get_runner(t_steps)
        concat_in = _concat_inputs(runner, x_full, tables)
        res = runner(concat_in)
        out = res["tags"].reshape(B, t_steps)
    except Exception:
        out = _run_spmd_fallback(t_steps, x_full, tables)
    return out.astype(np.int32)


kernel.last_exec_time_ns = None
